# revision 15
# baseline (speedup 1.0000x reference)
"""AtomConv (GCN message passing) distributed Bass kernel for 8 TRN2 NeuronCores.

out = relu(D^-1/2 (A+I) D^-1/2 (atom @ W.T + b)),  A = 3.2M random edges over 100K nodes.

Sharding (per the dst-routing hint): nodes 12500/core, edges routed to the core
owning the destination, weights replicated. Aggregation runs in 6-dim input
space: z[r] = [atom[r]*dis[r], dis[r]]; agg[c] = sum_{r->c} z[r];
out[c] = relu((dis[c]*agg[c]) @ [W|b].T).

Device mechanism: the per-edge gather of z rows uses gpsimd dma_gather
(256B elements, int16 indices) from 4 src-quarter tables on 4 SWDGE queues.
Slot grids are degree-sorted per (core, quarter) so grid columns are dense;
grid shapes are cross-core-maxed templates so one SPMD graph serves all 8
cores. Quarter partials are re-aligned via 3 more gathers, then a DVE matvec
(6->16) + relu finishes on device. Host work is routing/layout only
(bincount, sort, index packing) plus the final row unpermute/concat.
"""

import os
import numpy as np

N_NODES = 100000
N_IN = 5
N_OUT = 16
N_CORES = 8
NPC = N_NODES // N_CORES            # 12500
NQ = 4
QSZ = N_NODES // NQ                 # 25000 (int16-safe index range)
ES = 64                             # gather element = 64 f32 = 256B
P = 128
NPC_PAD = ((NPC + P - 1) // P) * P  # 12544
CHUNKS = NPC_PAD // P               # 98
QROWS = QSZ + 200                   # per-quarter z-table rows (row 0 = zeros)
AW = (QSZ + P - 1) // P             # 196 atom-wrap cols
MAX_CALL = 8192                     # slots per dma_gather call

LAST_EXEC_NS = None


def _host_prepare(atom, edge_index, W, b):
    src = np.asarray(edge_index[0]).astype(np.int64)
    dst = np.asarray(edge_index[1]).astype(np.int64)
    # deg includes the self loop; self-loop messages are added directly on
    # device (no gather slot needed)
    deg = (np.bincount(dst, minlength=N_NODES) + 1.0).astype(np.float32)

    core_of = dst // NPC
    quarter = src // QSZ

    per = {}
    dq = np.zeros((N_CORES, NQ, NPC_PAD), np.int64)
    for ci in range(N_CORES):
        mc = core_of == ci
        s_c, d_c, q_c = src[mc], dst[mc] - ci * NPC, quarter[mc]
        for q in range(NQ):
            mq = q_c == q
            d_loc = d_c[mq]
            s_loc = s_c[mq] - q * QSZ
            per[(ci, q)] = (d_loc, s_loc)
            dq[ci, q, :NPC] = np.bincount(d_loc, minlength=NPC)

    pi = np.argsort(dq, axis=2, kind="stable")          # ascending degree
    dq_sorted = np.take_along_axis(dq, pi, axis=2)
    K = dq_sorted.reshape(N_CORES, NQ, CHUNKS, P).max(axis=3).max(axis=0)
    K = np.maximum(K, 1).astype(np.int64)               # [NQ, CHUNKS] template

    idx_feeds = []
    for ci in range(N_CORES):
        q_feeds = []
        for q in range(NQ):
            d_loc, s_loc = per[(ci, q)]
            counts = dq[ci, q, :NPC]
            Kq = int(K[q].max())
            mat = np.zeros((NPC_PAD, Kq), np.int16)
            if len(d_loc):
                order = np.argsort(d_loc, kind="stable")
                d_sorted, s_sorted = d_loc[order], s_loc[order]
                starts = np.zeros(NPC, np.int64)
                starts[1:] = np.cumsum(counts)[:-1]
                kk = np.arange(len(d_sorted)) - starts[d_sorted]
                mat[d_sorted, kk] = (s_sorted + 1).astype(np.int16)
            g = mat[pi[ci, q]]
            slots = [
                g[c * P:(c + 1) * P, : int(K[q, c])].T.reshape(-1)
                for c in range(CHUNKS)
            ]
            q_feeds.append(np.concatenate(slots))
        idx_feeds.append(q_feeds)

    comb_feeds = []
    for ci in range(N_CORES):
        inv = np.empty((NQ, NPC_PAD), np.int64)
        for q in range(NQ):
            inv[q, pi[ci, q]] = np.arange(NPC_PAD)
        comb_feeds.append([inv[q][pi[ci, 0]].astype(np.int16) for q in range(1, NQ)])

    deg_pi0 = []
    atom_pi0 = []
    for ci in range(N_CORES):
        dpc = np.zeros(NPC_PAD, np.float32)
        dpc[:NPC] = deg[ci * NPC:(ci + 1) * NPC]
        dpc = np.maximum(dpc[pi[ci, 0]], 1.0)
        deg_pi0.append(np.ascontiguousarray(dpc.reshape(CHUNKS, P).T))  # [P, CH]
        apc = np.zeros((NPC_PAD, N_IN), np.float32)
        apc[:NPC] = np.asarray(atom, np.float32)[ci * NPC:(ci + 1) * NPC]
        apc = apc[pi[ci, 0]].reshape(CHUNKS, P, N_IN)
        atom_pi0.append(np.ascontiguousarray(apc.transpose(1, 0, 2)))  # [P, CH, 5]

    a = np.asarray(atom, np.float32)
    atom_q = np.zeros((NQ, P, AW, N_IN), np.float32)
    deg_q = np.ones((NQ, P, AW), np.float32)
    for q in range(NQ):
        blk = np.zeros((P * AW, N_IN), np.float32)
        blk[:QSZ] = a[q * QSZ:(q + 1) * QSZ]
        atom_q[q] = blk.reshape(P, AW, N_IN)
        dblk = np.ones(P * AW, np.float32)
        dblk[:QSZ] = np.maximum(deg[q * QSZ:(q + 1) * QSZ], 1.0)
        deg_q[q] = dblk.reshape(P, AW)

    W_ext = np.zeros((N_OUT, 6), np.float64)
    W_ext[:, :5] = np.asarray(W, np.float64)
    W_ext[:, 5] = np.asarray(b, np.float64)

    return dict(K=K, pi=pi, idx_feeds=idx_feeds, comb_feeds=comb_feeds,
                deg_pi0=deg_pi0, atom_pi0=atom_pi0, atom_q=atom_q, deg_q=deg_q,
                W_ext=W_ext)


def _wrap16(flat):
    """idx j -> sbuf (j%16, j//16), replicated across the 8 q7 cores."""
    n = len(flat)
    w = flat.reshape(n // 16, 16).T
    return np.ascontiguousarray(np.tile(w, (8, 1)).astype(np.int16))


def _plan_calls(K):
    plans = []
    for q in range(NQ):
        calls, cur, cur_slots, off = [], [], 0, 0
        for c in range(CHUNKS):
            s = int(K[q, c]) * P
            if cur_slots + s > MAX_CALL and cur:
                calls.append((off, cur_slots, cur))
                off += cur_slots
                cur, cur_slots = [], 0
            cur.append((c, int(K[q, c]), cur_slots))
            cur_slots += s
        if cur:
            calls.append((off, cur_slots, cur))
        plans.append(calls)
    return plans


def _build_graph(K, W_ext):
    import concourse.bass as bass
    import concourse.bacc as bacc
    import concourse.mybir as mybir
    import concourse.tile as tile
    from concourse import library_config

    f32 = mybir.dt.float32
    i16 = mybir.dt.int16
    AT = mybir.AluOpType
    AX = mybir.AxisListType

    plans = _plan_calls(K)
    S_q = [sum(int(K[q, c]) * P for c in range(CHUNKS)) for q in range(NQ)]

    nc = bacc.Bacc("TRN2", target_bir_lowering=False, debug=False,
                   num_swdge_queues=4)

    atom_in = nc.dram_tensor("atom_q", [NQ, P, AW, N_IN], f32, kind="ExternalInput")
    degq_in = nc.dram_tensor("deg_q", [NQ, P, AW], f32, kind="ExternalInput")
    degp_in = nc.dram_tensor("deg_pi0", [P, CHUNKS], f32, kind="ExternalInput")
    atomp_in = nc.dram_tensor("atom_pi0", [P, CHUNKS, N_IN], f32, kind="ExternalInput")
    wrep_in = nc.dram_tensor("w_rep", [P, 6 * N_OUT], f32, kind="ExternalInput")
    idx_ins = [nc.dram_tensor(f"idx_q{q}", [P, S_q[q] // 16], i16, kind="ExternalInput")
               for q in range(NQ)]
    comb_ins = [nc.dram_tensor(f"comb_q{q}", [P, NPC_PAD // 16], i16, kind="ExternalInput")
                for q in range(1, NQ)]
    out_t = nc.dram_tensor("out", [NPC_PAD, N_OUT], f32, kind="ExternalOutput")

    z_dram = nc.dram_tensor("z_tab", [NQ, QROWS, ES], f32, kind="Internal")
    part_dram = nc.dram_tensor("part", [NQ - 1, NPC_PAD, ES], f32, kind="Internal")

    with tile.TileContext(nc) as tc:
        with tc.tile_pool(name="sb", bufs=1) as pool, \
             tc.tile_pool(name="gp", bufs=5) as gpool, \
             tc.tile_pool(name="ip", bufs=4) as ipool, \
             tc.tile_pool(name="cp", bufs=2) as cpool, \
             tc.tile_pool(name="aq", bufs=2) as aqpool:
            nc.gpsimd.load_library(library_config.mlp)

            # ---- z tables: rows 1+p*AW+c <- [atom*dis | dis], row 0 <- 0
            zero64 = pool.tile([1, ES], f32)
            nc.vector.memset(zero64[:], 0.0)
            for q in range(NQ):
                nc.sync.dma_start(out=z_dram[q, 0:1, :], in_=zero64[:])
            G = 4
            GW = AW // G  # 49 rows per column-group
            for q in [1, 2, 3, 0]:
                at = pool.tile([P, AW * N_IN], f32, tag="at")
                dg = pool.tile([P, AW], f32, tag="dg")
                ds = pool.tile([P, AW], f32, tag="ds")
                nc.sync.dma_start(out=at[:], in_=atom_in[q].rearrange("p a f -> p (a f)"))
                nc.sync.dma_start(out=dg[:], in_=degq_in[q])
                nc.vector.reciprocal(ds[:], dg[:])
                nc.scalar.activation(ds[:], ds[:], mybir.ActivationFunctionType.Sqrt)
                atv = at[:].rearrange("p (a f) -> p a f", f=N_IN)
                for g in range(G):
                    # 64-wide staging tile: cols 0:6 real, 6:64 garbage (never
                    # read back -- the gather extract only touches cols 0:6)
                    zb = pool.tile([P, GW * ES], f32, tag="zb64")
                    zbv = zb[:].rearrange("p (a e) -> p a e", e=ES)
                    sl = slice(g * GW, (g + 1) * GW)
                    for f in range(N_IN):
                        nc.vector.tensor_tensor(zbv[:, :, f], atv[:, sl, f], ds[:, sl], op=AT.mult)
                    nc.vector.tensor_copy(zbv[:, :, 5], ds[:, sl])
                    nc.sync.dma_start(
                        out=bass.AP(z_dram, q * QROWS * ES + (1 + g * GW) * ES,
                                    [[AW * ES, P], [1, GW * ES]]),
                        in_=zb[:],
                    )

            acc = pool.tile([P, CHUNKS * 6], f32)
            accv = acc[:].rearrange("p (c f) -> p c f", f=6)
            nc.vector.memset(acc[:], 0.0)

            # dis_dst and self-loop term have no gather deps: compute early
            dgp = pool.tile([P, CHUNKS], f32)
            dsp = pool.tile([P, CHUNKS], f32)
            nc.sync.dma_start(out=dgp[:], in_=degp_in.ap())
            nc.vector.reciprocal(dsp[:], dgp[:])
            nc.scalar.activation(dsp[:], dsp[:], mybir.ActivationFunctionType.Sqrt)
            sl6 = pool.tile([P, CHUNKS * 6], f32)
            sl6v = sl6[:].rearrange("p (c f) -> p c f", f=6)
            ap0 = pool.tile([P, CHUNKS * N_IN], f32, tag="ap0")
            nc.sync.dma_start(out=ap0[:], in_=atomp_in.ap().rearrange("p c f -> p (c f)"))
            ap0v = ap0[:].rearrange("p (c f) -> p c f", f=N_IN)
            for f in range(N_IN):
                nc.vector.tensor_tensor(sl6v[:, :, f], ap0v[:, :, f], dsp[:], op=AT.mult)
            nc.vector.tensor_copy(sl6v[:, :, 5], dsp[:])
            accq = None

            qn = 0
            for q in [1, 2, 3, 0]:
                if q > 0:
                    accq = aqpool.tile([P, CHUNKS * 6], f32, tag="accq")
                tgt = accv if q == 0 else accq[:].rearrange("p (c f) -> p c f", f=6)
                for (off, nslots, chunks) in plans[q]:
                    it = ipool.tile([P, MAX_CALL // 16], i16, tag="idx")
                    nc.sync.dma_start(
                        out=it[:, : nslots // 16],
                        in_=idx_ins[q][:, off // 16:(off + nslots) // 16])
                    gb = gpool.tile([P, (MAX_CALL // P) * ES], f32, tag="gb")
                    gbv = gb[:].rearrange("p (m d) -> p m d", m=MAX_CALL // P)
                    nc.gpsimd.dma_gather(
                        out_ap=gbv[:, : nslots // P, :],
                        in_ap=z_dram[q],
                        idxs_ap=it[:, : nslots // 16],
                        num_idxs=nslots,
                        num_idxs_reg=nslots,
                        elem_size=ES,
                        single_packet=False,
                        queue_num=qn % 4,
                    )
                    qn += 1
                    for (c, kc, loff) in chunks:
                        col0 = loff // P
                        src = gbv[:, col0:col0 + kc, 0:6].rearrange("p m d -> p d m")
                        if q == 0:
                            t6 = gpool.tile([P, 6], f32, tag="t6")
                            nc.vector.tensor_reduce(t6[:], src, axis=AX.X, op=AT.add)
                            nc.vector.tensor_tensor(tgt[:, c, :], tgt[:, c, :], t6[:], op=AT.add)
                        else:
                            nc.vector.tensor_reduce(tgt[:, c, :], src, axis=AX.X, op=AT.add)
                if q > 0:
                    # stage pi_q-ordered partial to DRAM rows r=c*128+p, cols 0:6
                    nc.sync.dma_start(
                        out=bass.AP(part_dram, (q - 1) * NPC_PAD * ES,
                                    [[ES, P], [P * ES, CHUNKS], [1, 6]]),
                        in_=tgt,
                    )
                    # combine: gather this quarter's partial into pi_0 order
                    ct = ipool.tile([P, NPC_PAD // 16], i16, tag="cidx")
                    nc.sync.dma_start(out=ct[:], in_=comb_ins[q - 1].ap())
                    gc = cpool.tile([P, CHUNKS * ES], f32, tag="gc")
                    gcv = gc[:].rearrange("p (m d) -> p m d", m=CHUNKS)
                    nc.gpsimd.dma_gather(
                        out_ap=gcv,
                        in_ap=part_dram[q - 1],
                        idxs_ap=ct[:],
                        num_idxs=NPC_PAD,
                        num_idxs_reg=NPC_PAD,
                        elem_size=ES,
                        single_packet=False,
                        queue_num=(qn + 2) % 4,
                    )
                    nc.vector.tensor_tensor(accv, accv, gcv[:, :, 0:6], op=AT.add)

            # ---- finish: add self-loop term, dis_dst scale, 6->16 matvec, relu
            nc.vector.tensor_tensor(acc[:], acc[:], sl6[:], op=AT.add)
            for f in range(6):
                nc.vector.tensor_tensor(accv[:, :, f], accv[:, :, f], dsp[:], op=AT.mult)

            wr = pool.tile([P, 6 * N_OUT], f32, tag="wr")
            nc.sync.dma_start(out=wr[:], in_=wrep_in.ap())
            wrv = wr[:].rearrange("p (f o) -> p f o", o=N_OUT)
            o16 = pool.tile([P, CHUNKS * N_OUT], f32)
            o16v = o16[:].rearrange("p (c o) -> p c o", o=N_OUT)
            t16 = pool.tile([P, CHUNKS * N_OUT], f32, tag="t16")
            t16v = t16[:].rearrange("p (c o) -> p c o", o=N_OUT)
            for f in range(6):
                a_b = accv[:, :, f:f + 1].to_broadcast([P, CHUNKS, N_OUT])
                w_b = wrv[:, f:f + 1, :].to_broadcast([P, CHUNKS, N_OUT])
                if f == 0:
                    nc.vector.tensor_tensor(o16v, a_b, w_b, op=AT.mult)
                else:
                    nc.vector.tensor_tensor(t16v, a_b, w_b, op=AT.mult)
                    nc.vector.tensor_tensor(o16v, o16v, t16v, op=AT.add)
            nc.vector.tensor_scalar_max(o16[:], o16[:], 0.0)

            # out rows r=c*128+p
            nc.sync.dma_start(
                out=bass.AP(out_t, 0, [[N_OUT, P], [P * N_OUT, CHUNKS], [1, N_OUT]]),
                in_=o16v,
            )

    nc.compile()
    return nc


def kernel(**inputs):
    global LAST_EXEC_NS
    atom = inputs["atom"]
    edge_index = inputs["edge_index"]
    W = inputs["W"]
    b = inputs["b"]

    prep = _host_prepare(atom, edge_index, W, b)
    nc = _build_graph(prep["K"], prep["W_ext"])

    from concourse import bass_utils

    in_maps = []
    for ci in range(N_CORES):
        m = {
            "atom_q": prep["atom_q"],
            "deg_q": prep["deg_q"],
            "deg_pi0": prep["deg_pi0"][ci],
            "atom_pi0": prep["atom_pi0"][ci],
            "w_rep": np.ascontiguousarray(
                np.tile(prep["W_ext"].astype(np.float32).T.reshape(1, 6 * N_OUT),
                        (P, 1))),
            "out": np.zeros((NPC_PAD, N_OUT), np.float32),
        }
        for q in range(NQ):
            m[f"idx_q{q}"] = _wrap16(prep["idx_feeds"][ci][q])
        for q in range(1, NQ):
            m[f"comb_q{q}"] = _wrap16(prep["comb_feeds"][ci][q - 1])
        m.pop("out")
        in_maps.append(m)

    trace = bool(os.environ.get("KERNEL_TRACE"))
    if trace:
        try:
            import tracing_shim
            tracing_shim.install()
        except Exception:
            trace = False

    res = bass_utils.run_bass_kernel_spmd(
        nc, in_maps, core_ids=list(range(N_CORES)), trace=trace
    )
    LAST_EXEC_NS = res.exec_time_ns

    out = np.empty((N_NODES, N_OUT), np.float32)
    for ci in range(N_CORES):
        rows = res.results[ci]["out"]  # [NPC_PAD, 16], row j -> node pi0[j]
        pi0 = prep["pi"][ci, 0]
        real = pi0 < NPC
        out[ci * NPC + pi0[real]] = rows[real]
    return out


# revision 16
# speedup vs baseline: 1.0714x; 1.0714x over previous
"""AtomConv (GCN message passing) distributed Bass kernel for 8 TRN2 NeuronCores.

out = relu(D^-1/2 (A+I) D^-1/2 (atom @ W.T + b)),  A = 3.2M random edges over 100K nodes.

Sharding (per the dst-routing hint): nodes 12500/core, edges routed to the core
owning the destination, weights replicated. Aggregation runs in 6-dim input
space: z[r] = [atom[r]*dis[r], dis[r]]; agg[c] = sum_{r->c} z[r];
out[c] = relu((dis[c]*agg[c]) @ [W|b].T).

Device mechanism: the per-edge gather of z rows uses gpsimd dma_gather
(256B elements, int16 indices) from 4 src-quarter tables on 4 SWDGE queues.
Slot grids are degree-sorted per (core, quarter) so grid columns are dense;
grid shapes are cross-core-maxed templates so one SPMD graph serves all 8
cores. Quarter partials are re-aligned via 3 more gathers, then a DVE matvec
(6->16) + relu finishes on device. Host work is routing/layout only
(bincount, sort, index packing) plus the final row unpermute/concat.
"""

import os
import numpy as np

N_NODES = 100000
N_IN = 5
N_OUT = 16
N_CORES = 8
NPC = N_NODES // N_CORES            # 12500
NQ = 4
QSZ = N_NODES // NQ                 # 25000 (int16-safe index range)
ES = 64                             # gather element = 64 f32 = 256B
P = 128
NPC_PAD = ((NPC + P - 1) // P) * P  # 12544
CHUNKS = NPC_PAD // P               # 98
QROWS = QSZ + 200                   # per-quarter z-table rows (row 0 = zeros)
AW = (QSZ + P - 1) // P             # 196 atom-wrap cols
MAX_CALL = 8192                     # slots per dma_gather call

LAST_EXEC_NS = None


def _host_prepare(atom, edge_index, W, b):
    src = np.asarray(edge_index[0]).astype(np.int64)
    dst = np.asarray(edge_index[1]).astype(np.int64)
    # deg includes the self loop; self-loop messages are added directly on
    # device (no gather slot needed)
    deg = (np.bincount(dst, minlength=N_NODES) + 1.0).astype(np.float32)

    core_of = dst // NPC
    quarter = src // QSZ

    per = {}
    dq = np.zeros((N_CORES, NQ, NPC_PAD), np.int64)
    for ci in range(N_CORES):
        mc = core_of == ci
        s_c, d_c, q_c = src[mc], dst[mc] - ci * NPC, quarter[mc]
        for q in range(NQ):
            mq = q_c == q
            d_loc = d_c[mq]
            s_loc = s_c[mq] - q * QSZ
            per[(ci, q)] = (d_loc, s_loc)
            dq[ci, q, :NPC] = np.bincount(d_loc, minlength=NPC)

    pi = np.argsort(dq, axis=2, kind="stable")          # ascending degree
    dq_sorted = np.take_along_axis(dq, pi, axis=2)
    K = dq_sorted.reshape(N_CORES, NQ, CHUNKS, P).max(axis=3).max(axis=0)
    K = np.maximum(K, 1).astype(np.int64)               # [NQ, CHUNKS] template

    idx_feeds = []
    for ci in range(N_CORES):
        q_feeds = []
        for q in range(NQ):
            d_loc, s_loc = per[(ci, q)]
            counts = dq[ci, q, :NPC]
            Kq = int(K[q].max())
            mat = np.zeros((NPC_PAD, Kq), np.int16)
            if len(d_loc):
                order = np.argsort(d_loc, kind="stable")
                d_sorted, s_sorted = d_loc[order], s_loc[order]
                starts = np.zeros(NPC, np.int64)
                starts[1:] = np.cumsum(counts)[:-1]
                kk = np.arange(len(d_sorted)) - starts[d_sorted]
                mat[d_sorted, kk] = (s_sorted + 1).astype(np.int16)
            g = mat[pi[ci, q]]
            slots = [
                g[c * P:(c + 1) * P, : int(K[q, c])].T.reshape(-1)
                for c in range(CHUNKS)
            ]
            q_feeds.append(np.concatenate(slots))
        idx_feeds.append(q_feeds)

    comb_feeds = []
    for ci in range(N_CORES):
        inv = np.empty((NQ, NPC_PAD), np.int64)
        for q in range(NQ):
            inv[q, pi[ci, q]] = np.arange(NPC_PAD)
        comb_feeds.append([inv[q][pi[ci, 0]].astype(np.int16) for q in range(1, NQ)])

    deg_pi0 = []
    atom_pi0 = []
    for ci in range(N_CORES):
        dpc = np.zeros(NPC_PAD, np.float32)
        dpc[:NPC] = deg[ci * NPC:(ci + 1) * NPC]
        dpc = np.maximum(dpc[pi[ci, 0]], 1.0)
        deg_pi0.append(np.ascontiguousarray(dpc.reshape(CHUNKS, P).T))  # [P, CH]
        apc = np.zeros((NPC_PAD, N_IN), np.float32)
        apc[:NPC] = np.asarray(atom, np.float32)[ci * NPC:(ci + 1) * NPC]
        apc = apc[pi[ci, 0]].reshape(CHUNKS, P, N_IN)
        atom_pi0.append(np.ascontiguousarray(apc.transpose(1, 0, 2)))  # [P, CH, 5]

    a = np.asarray(atom, np.float32)
    atom_q = np.zeros((NQ, P, AW, N_IN), np.float32)
    deg_q = np.ones((NQ, P, AW), np.float32)
    for q in range(NQ):
        blk = np.zeros((P * AW, N_IN), np.float32)
        blk[:QSZ] = a[q * QSZ:(q + 1) * QSZ]
        atom_q[q] = blk.reshape(P, AW, N_IN)
        dblk = np.ones(P * AW, np.float32)
        dblk[:QSZ] = np.maximum(deg[q * QSZ:(q + 1) * QSZ], 1.0)
        deg_q[q] = dblk.reshape(P, AW)

    W_ext = np.zeros((N_OUT, 6), np.float64)
    W_ext[:, :5] = np.asarray(W, np.float64)
    W_ext[:, 5] = np.asarray(b, np.float64)

    return dict(K=K, pi=pi, idx_feeds=idx_feeds, comb_feeds=comb_feeds,
                deg_pi0=deg_pi0, atom_pi0=atom_pi0, atom_q=atom_q, deg_q=deg_q,
                W_ext=W_ext)


def _wrap16(flat):
    """idx j -> sbuf (j%16, j//16), replicated across the 8 q7 cores."""
    n = len(flat)
    w = flat.reshape(n // 16, 16).T
    return np.ascontiguousarray(np.tile(w, (8, 1)).astype(np.int16))


def _plan_calls(K):
    plans = []
    for q in range(NQ):
        calls, cur, cur_slots, off = [], [], 0, 0
        for c in range(CHUNKS):
            s = int(K[q, c]) * P
            if cur_slots + s > MAX_CALL and cur:
                calls.append((off, cur_slots, cur))
                off += cur_slots
                cur, cur_slots = [], 0
            cur.append((c, int(K[q, c]), cur_slots))
            cur_slots += s
        if cur:
            calls.append((off, cur_slots, cur))
        plans.append(calls)
    return plans


def _build_graph(K, W_ext):
    import concourse.bass as bass
    import concourse.bacc as bacc
    import concourse.mybir as mybir
    import concourse.tile as tile
    from concourse import library_config

    f32 = mybir.dt.float32
    i16 = mybir.dt.int16
    AT = mybir.AluOpType
    AX = mybir.AxisListType

    plans = _plan_calls(K)
    S_q = [sum(int(K[q, c]) * P for c in range(CHUNKS)) for q in range(NQ)]

    nc = bacc.Bacc("TRN2", target_bir_lowering=False, debug=False,
                   num_swdge_queues=4)

    atom_in = nc.dram_tensor("atom_q", [NQ, P, AW, N_IN], f32, kind="ExternalInput")
    degq_in = nc.dram_tensor("deg_q", [NQ, P, AW], f32, kind="ExternalInput")
    degp_in = nc.dram_tensor("deg_pi0", [P, CHUNKS], f32, kind="ExternalInput")
    atomp_in = nc.dram_tensor("atom_pi0", [P, CHUNKS, N_IN], f32, kind="ExternalInput")
    wrep_in = nc.dram_tensor("w_rep", [P, 6 * N_OUT], f32, kind="ExternalInput")
    idx_ins = [nc.dram_tensor(f"idx_q{q}", [P, S_q[q] // 16], i16, kind="ExternalInput")
               for q in range(NQ)]
    comb_ins = [nc.dram_tensor(f"comb_q{q}", [P, NPC_PAD // 16], i16, kind="ExternalInput")
                for q in range(1, NQ)]
    out_t = nc.dram_tensor("out", [NPC_PAD, N_OUT], f32, kind="ExternalOutput")

    z_dram = nc.dram_tensor("z_tab", [NQ, QROWS, ES], f32, kind="Internal")
    part_dram = nc.dram_tensor("part", [NQ - 1, NPC_PAD, ES], f32, kind="Internal")

    with tile.TileContext(nc) as tc:
        with tc.tile_pool(name="sb", bufs=1) as pool, \
             tc.tile_pool(name="gp", bufs=4) as gpool, \
             tc.tile_pool(name="ip", bufs=4) as ipool, \
             tc.tile_pool(name="cp", bufs=2) as cpool, \
             tc.tile_pool(name="aq", bufs=2) as aqpool:
            nc.gpsimd.load_library(library_config.mlp)

            # ---- z tables: rows 1+p*AW+c <- [atom*dis | dis], row 0 <- 0
            zero64 = pool.tile([1, ES], f32)
            nc.vector.memset(zero64[:], 0.0)
            for q in range(NQ):
                nc.sync.dma_start(out=z_dram[q, 0:1, :], in_=zero64[:])
            G = 4
            GW = AW // G  # 49 rows per column-group
            for q in [1, 2, 3, 0]:
                at = pool.tile([P, AW * N_IN], f32, tag="at")
                dg = pool.tile([P, AW], f32, tag="dg")
                ds = pool.tile([P, AW], f32, tag="ds")
                nc.sync.dma_start(out=at[:], in_=atom_in[q].rearrange("p a f -> p (a f)"))
                nc.sync.dma_start(out=dg[:], in_=degq_in[q])
                nc.vector.reciprocal(ds[:], dg[:])
                nc.scalar.activation(ds[:], ds[:], mybir.ActivationFunctionType.Sqrt)
                atv = at[:].rearrange("p (a f) -> p a f", f=N_IN)
                for g in range(G):
                    # 64-wide staging tile: cols 0:6 real, 6:64 garbage (never
                    # read back -- the gather extract only touches cols 0:6)
                    zb = pool.tile([P, GW * ES], f32, tag="zb64")
                    zbv = zb[:].rearrange("p (a e) -> p a e", e=ES)
                    sl = slice(g * GW, (g + 1) * GW)
                    for f in range(N_IN):
                        nc.vector.tensor_tensor(zbv[:, :, f], atv[:, sl, f], ds[:, sl], op=AT.mult)
                    nc.vector.tensor_copy(zbv[:, :, 5], ds[:, sl])
                    nc.sync.dma_start(
                        out=bass.AP(z_dram, q * QROWS * ES + (1 + g * GW) * ES,
                                    [[AW * ES, P], [1, GW * ES]]),
                        in_=zb[:],
                    )

            acc = pool.tile([P, CHUNKS * 6], f32)
            accv = acc[:].rearrange("p (c f) -> p c f", f=6)
            nc.vector.memset(acc[:], 0.0)

            # dis_dst and self-loop term have no gather deps: compute early
            dgp = pool.tile([P, CHUNKS], f32)
            dsp = pool.tile([P, CHUNKS], f32)
            nc.sync.dma_start(out=dgp[:], in_=degp_in.ap())
            nc.vector.reciprocal(dsp[:], dgp[:])
            nc.scalar.activation(dsp[:], dsp[:], mybir.ActivationFunctionType.Sqrt)
            sl6 = pool.tile([P, CHUNKS * 6], f32)
            sl6v = sl6[:].rearrange("p (c f) -> p c f", f=6)
            ap0 = pool.tile([P, CHUNKS * N_IN], f32, tag="ap0")
            nc.sync.dma_start(out=ap0[:], in_=atomp_in.ap().rearrange("p c f -> p (c f)"))
            ap0v = ap0[:].rearrange("p (c f) -> p c f", f=N_IN)
            for f in range(N_IN):
                nc.vector.tensor_tensor(sl6v[:, :, f], ap0v[:, :, f], dsp[:], op=AT.mult)
            nc.vector.tensor_copy(sl6v[:, :, 5], dsp[:])
            accq = None

            qn = 0
            for q in [1, 2, 3, 0]:
                if q > 0:
                    accq = aqpool.tile([P, CHUNKS * 6], f32, tag="accq")
                tgt = accv if q == 0 else accq[:].rearrange("p (c f) -> p c f", f=6)
                for (off, nslots, chunks) in plans[q]:
                    it = ipool.tile([P, MAX_CALL // 16], i16, tag="idx")
                    nc.sync.dma_start(
                        out=it[:, : nslots // 16],
                        in_=idx_ins[q][:, off // 16:(off + nslots) // 16])
                    gb = gpool.tile([P, (MAX_CALL // P) * ES], f32, tag="gb")
                    gbv = gb[:].rearrange("p (m d) -> p m d", m=MAX_CALL // P)
                    nc.gpsimd.dma_gather(
                        out_ap=gbv[:, : nslots // P, :],
                        in_ap=z_dram[q],
                        idxs_ap=it[:, : nslots // 16],
                        num_idxs=nslots,
                        num_idxs_reg=nslots,
                        elem_size=ES,
                        single_packet=False,
                        queue_num=qn % 4,
                    )
                    qn += 1
                    for (c, kc, loff) in chunks:
                        col0 = loff // P
                        src = gbv[:, col0:col0 + kc, 0:6].rearrange("p m d -> p d m")
                        if q == 0:
                            t6 = gpool.tile([P, 6], f32, tag="t6")
                            nc.vector.tensor_reduce(t6[:], src, axis=AX.X, op=AT.add)
                            nc.vector.tensor_tensor(tgt[:, c, :], tgt[:, c, :], t6[:], op=AT.add)
                        else:
                            nc.vector.tensor_reduce(tgt[:, c, :], src, axis=AX.X, op=AT.add)
                if q > 0:
                    # stage pi_q-ordered partial to DRAM rows r=c*128+p, cols 0:6
                    nc.sync.dma_start(
                        out=bass.AP(part_dram, (q - 1) * NPC_PAD * ES,
                                    [[ES, P], [P * ES, CHUNKS], [1, 6]]),
                        in_=tgt,
                    )
                    # combine: gather this quarter's partial into pi_0 order
                    ct = ipool.tile([P, NPC_PAD // 16], i16, tag="cidx")
                    nc.sync.dma_start(out=ct[:], in_=comb_ins[q - 1].ap())
                    gc = cpool.tile([P, CHUNKS * ES], f32, tag="gc")
                    gcv = gc[:].rearrange("p (m d) -> p m d", m=CHUNKS)
                    nc.gpsimd.dma_gather(
                        out_ap=gcv,
                        in_ap=part_dram[q - 1],
                        idxs_ap=ct[:],
                        num_idxs=NPC_PAD,
                        num_idxs_reg=NPC_PAD,
                        elem_size=ES,
                        single_packet=False,
                        queue_num=(qn + 2) % 4,
                    )
                    nc.vector.tensor_tensor(accv, accv, gcv[:, :, 0:6], op=AT.add)

            # ---- finish: add self-loop term, dis_dst scale, 6->16 matvec, relu
            nc.vector.tensor_tensor(acc[:], acc[:], sl6[:], op=AT.add)
            for f in range(6):
                nc.vector.tensor_tensor(accv[:, :, f], accv[:, :, f], dsp[:], op=AT.mult)

            wr = pool.tile([P, 6 * N_OUT], f32, tag="wr")
            nc.sync.dma_start(out=wr[:], in_=wrep_in.ap())
            wrv = wr[:].rearrange("p (f o) -> p f o", o=N_OUT)
            o16 = pool.tile([P, CHUNKS * N_OUT], f32)
            o16v = o16[:].rearrange("p (c o) -> p c o", o=N_OUT)
            t16 = pool.tile([P, CHUNKS * N_OUT], f32, tag="t16")
            t16v = t16[:].rearrange("p (c o) -> p c o", o=N_OUT)
            for f in range(6):
                a_b = accv[:, :, f:f + 1].to_broadcast([P, CHUNKS, N_OUT])
                w_b = wrv[:, f:f + 1, :].to_broadcast([P, CHUNKS, N_OUT])
                if f == 0:
                    nc.vector.tensor_tensor(o16v, a_b, w_b, op=AT.mult)
                else:
                    nc.vector.tensor_tensor(t16v, a_b, w_b, op=AT.mult)
                    nc.vector.tensor_tensor(o16v, o16v, t16v, op=AT.add)
            nc.vector.tensor_scalar_max(o16[:], o16[:], 0.0)

            # out rows r=c*128+p
            nc.sync.dma_start(
                out=bass.AP(out_t, 0, [[N_OUT, P], [P * N_OUT, CHUNKS], [1, N_OUT]]),
                in_=o16v,
            )

    nc.compile()
    return nc


def kernel(**inputs):
    global LAST_EXEC_NS
    atom = inputs["atom"]
    edge_index = inputs["edge_index"]
    W = inputs["W"]
    b = inputs["b"]

    prep = _host_prepare(atom, edge_index, W, b)
    nc = _build_graph(prep["K"], prep["W_ext"])

    from concourse import bass_utils

    in_maps = []
    for ci in range(N_CORES):
        m = {
            "atom_q": prep["atom_q"],
            "deg_q": prep["deg_q"],
            "deg_pi0": prep["deg_pi0"][ci],
            "atom_pi0": prep["atom_pi0"][ci],
            "w_rep": np.ascontiguousarray(
                np.tile(prep["W_ext"].astype(np.float32).T.reshape(1, 6 * N_OUT),
                        (P, 1))),
            "out": np.zeros((NPC_PAD, N_OUT), np.float32),
        }
        for q in range(NQ):
            m[f"idx_q{q}"] = _wrap16(prep["idx_feeds"][ci][q])
        for q in range(1, NQ):
            m[f"comb_q{q}"] = _wrap16(prep["comb_feeds"][ci][q - 1])
        m.pop("out")
        in_maps.append(m)

    trace = bool(os.environ.get("KERNEL_TRACE"))
    if trace:
        try:
            import tracing_shim
            tracing_shim.install()
        except Exception:
            trace = False

    res = bass_utils.run_bass_kernel_spmd(
        nc, in_maps, core_ids=list(range(N_CORES)), trace=trace
    )
    LAST_EXEC_NS = res.exec_time_ns

    out = np.empty((N_NODES, N_OUT), np.float32)
    for ci in range(N_CORES):
        rows = res.results[ci]["out"]  # [NPC_PAD, 16], row j -> node pi0[j]
        pi0 = prep["pi"][ci, 0]
        real = pi0 < NPC
        out[ci * NPC + pi0[real]] = rows[real]
    return out
